# revision 22
# baseline (speedup 1.0000x reference)
"""DebiasedPosLossV2 contrastive loss on 8 Trainium2 NeuronCores.

Math (reference, B=4096, D=128, TEMP=0.5, TAU=0.1):
    out = concat([out_1, out_2])            # [2B, D], rows L2-normalized
    ez  = exp(out @ out.T / TEMP)           # [2B, 2B], symmetric
    full_i = sum_j ez_ij
    S_i    = sum_{j: t_j == t_i} ez_ij      (t = concat([target, target]))
    Ng = full - S;  loss = mean(log(o2) - log(o1))

ez is symmetric, so only the upper block-triangle is computed (plus the
block diagonal), cutting PE matmul and ScalarE exp work roughly in half
versus the full matrix.  For a computed element (i <= j):
  - its contribution to column j (full_j / S_j) comes from a one-hot
    class matmul on the TensorEngine (column side),
  - its contribution to ROW i (= the missing lower part of column i, by
    symmetry) comes from free-dim reductions on the otherwise-idle
    Vector/GpSimd engines: tensor_scalar with accum_out for full,
    scalar_tensor_tensor (mask-and-multiply-and-sum) for S.
The host sums the per-core column-side and row-side partials.

SPMD uniformity: run_bass_kernel_spmd runs ONE program on all 8 cores,
so every core must execute an identical instruction stream.  The upper
triangle (64x64 blocks of 128) is decomposed recursively:
  Q64:  rows  0-31 x col-blocks 32-63  -> each core a 512-col strip
  Q32:  rows  0-15 x blocks 16-31, rows 32-47 x blocks 48-63 -> 256 cols
  Q16:  4 instances of rows (8k..8k+7) x next 8 blocks -> 128 cols each
  octant: blocks 8c..8c+7 square, computed FULLY, column-side only
    (a symmetric square's column sums already count every ordered pair).
Every core gets: a 56-iteration main loop over global row-blocks 0..55
with a band-contiguous packed column buffer, plus an 8-iteration octant
loop.  All geometry is core-independent; only host-packed data differs.
Per-core work: 288 [128,128] blocks (vs 512 for the full matrix).
"""

import os
import sys

if "/opt/trn_rl_repo" not in sys.path:
    sys.path.insert(0, "/opt/trn_rl_repo")

_KDBG = set(os.environ.get("KDBG", "").split(","))

from contextlib import ExitStack

import numpy as np

import concourse.bass as bass
import concourse.mybir as mybir
import concourse.tile as tile
from concourse.bass_utils import run_bass_kernel_spmd

B = 4096
D = 128
TWO_B = 2 * B
TEMPERATURE = 0.5
TAU_PLUS = 0.1
N_CORES = 8
NCLS = 100
NSEG = 56 * 128          # main-loop rows (global rows 0..7167)
NCOL = 2560              # packed columns per core (1536 main + 1024 octant)

F16 = mybir.dt.float16
F32 = mybir.dt.float32

# main-loop segments: name -> (colbuf_lo, length, q_bank, bank_off, r_lo, r_hi)
SEGS = {
    "q16_1": (0, 128, "q16", 0, 0, 8),
    "q32_p": (128, 256, "q32", 0, 0, 16),
    "q64": (384, 512, "q64", 0, 0, 32),
    "q16_2": (896, 128, "q16", 128, 16, 24),
    "q16_3": (1024, 128, "q16", 256, 32, 40),
    "q32_r": (1152, 256, "q32", 256, 32, 48),
    "q16_4": (1408, 128, "q16", 384, 48, 56),
}

# A matmul with start=True resets the whole PSUM bank, so segments sharing
# a bank must be strictly sequential: windows of rows, each followed by the
# extract of any segment that just finished.  Within a window, rows are
# batched into <=1024-column exp groups.
WINDOWS = [
    ([[r] for r in range(0, 8)], ["q16_1"]),            # spans 896
    ([[r] for r in range(8, 16)], ["q32_p"]),           # spans 768
    ([[r] for r in range(16, 24)], ["q16_2"]),          # spans 640
    ([[24, 25], [26, 27], [28, 29], [30, 31]], ["q64"]),     # 512+512
    ([[32, 33], [34, 35], [36, 37], [38, 39]], ["q16_3"]),   # 384+384
    ([[40, 41, 42, 43], [44, 45, 46, 47]], ["q32_r"]),       # 256 x4
    ([list(range(48, 56))], ["q16_4"]),                      # 128 x8
]
BATCHES = [b for w, _ in WINDOWS for b in w]

_PROGRAM = None
_PROGRAM_SPLIT = False


def _active_segs(r):
    return [k for k, s in SEGS.items() if s[4] <= r < s[5]]


def _span(r):
    segs = [SEGS[k] for k in _active_segs(r)]
    lo = min(s[0] for s in segs)
    hi = max(s[0] + s[1] for s in segs)
    assert hi - lo == sum(s[1] for s in segs), f"span not contiguous at r={r}"
    return lo, hi


def _seg_first_last():
    """first/last processed row per segment, in BATCHES order."""
    order = [r for b in BATCHES for r in b]
    fl = {}
    for name, s in SEGS.items():
        rs = [r for r in order if s[4] <= r < s[5]]
        fl[name] = (rs[0], rs[-1])
    return fl


def _split512(a, b):
    """split [a,b) at 512 boundaries (PSUM bank bounds)."""
    out = []
    while a < b:
        nxt = min(b, (a // 512 + 1) * 512)
        out.append((a, nxt))
        a = nxt
    return out


def _build_program() -> bass.Bass:
    nc = bass.Bass()

    # boot: everything the octant phase needs, one DMA: [oct cols | oct oh]
    boot_d = nc.declare_dram_parameter("boot", [128, 2048], F16, isOutput=False)
    colsm_d = nc.declare_dram_parameter("colsm", [128, 1536], F16, isOutput=False)
    xm_d = nc.declare_dram_parameter("xm", [128, NSEG], F16, isOutput=False)
    cm_d = nc.declare_dram_parameter("cm", [NCLS + 1, NCOL], F32, isOutput=False)
    ohm_d = nc.declare_dram_parameter("ohm", [128, NSEG], F16, isOutput=False)
    tc_d = nc.declare_dram_parameter("tc", [128, NCOL], F16, isOutput=False)
    tr_d = nc.declare_dram_parameter("tr", [128, 56], F32, isOutput=False)
    fs_d = nc.declare_dram_parameter("fs", [1, 2 * NCOL], F32, isOutput=True)
    rw_d = nc.declare_dram_parameter("rw", [128, 112], F32, isOutput=True)

    fl = _seg_first_last()
    ALU = mybir.AluOpType

    with ExitStack() as ctx:
        tc_ctx = ctx.enter_context(tile.TileContext(nc))
        const = ctx.enter_context(tc_ctx.tile_pool(name="const", bufs=1))
        ezp = ctx.enter_context(tc_ctx.tile_pool(name="ez", bufs=3))
        zp = ctx.enter_context(tc_ctx.tile_pool(name="z", bufs=2, space="PSUM"))
        stp = ctx.enter_context(tc_ctx.tile_pool(name="st", bufs=1, space="PSUM"))

        # input DMAs, ordered by first use (one queue -> one semaphore)
        boot = const.tile([128, 2048], F16, tag="boot")
        nc.sync.dma_start(boot[:], boot_d[:])
        colsm = const.tile([128, 1536], F16, tag="colsm")
        nc.sync.dma_start(colsm[:], colsm_d[:])
        xm = const.tile([128, NSEG], F16, tag="xm")
        nc.sync.dma_start(xm[:], xm_d[:])
        cm = const.tile([NCLS + 1, NCOL], F32, tag="cm")
        nc.sync.dma_start(cm[:], cm_d[:])
        ohm = const.tile([128, NSEG], F16, tag="ohm")
        nc.sync.dma_start(ohm[:], ohm_d[:])
        tcb = const.tile([128, NCOL], F16, tag="tcb")
        nc.sync.dma_start(tcb[:], tc_d[:])
        trb = const.tile([128, 56], F32, tag="trb")
        nc.sync.dma_start(trb[:], tr_d[:])

        ones = const.tile([NCLS + 1, 1], F16, tag="ones")
        nc.gpsimd.memset(ones[:], 1.0)
        fs = const.tile([1, 2 * NCOL], F32, tag="fs")
        rw = const.tile([128, 112], F32, tag="rw")
        if _KDBG & {"norow", "nots", "nostt"}:
            nc.vector.memset(rw[:], 0.0)
        mkp = ctx.enter_context(tc_ctx.tile_pool(name="mkp", bufs=2))
        tch = const.tile([1, 8], F32, tag="tch")         # toucher scratch

        # warm the exp activation table while boot DMA streams
        nc.scalar.activation(
            tch[0:1, 4:5], ones[0:1, 0:1], mybir.ActivationFunctionType.Exp
        )
        # touchers: absorb DMA waits on cheap ops so the real consumers
        # carry a single sync wait each (walrus limit)
        nc.vector.tensor_copy(tch[0:1, 0:1], cm[0:1, 0:1])
        nc.vector.tensor_copy(tch[0:1, 1:2], tcb[0:1, 0:1])
        nc.vector.tensor_copy(tch[0:1, 2:3], trb[0:1, 0:1])

        def extract(q, bank_off, colbuf_lo, length):
            """colQ bank slice -> fs[full | S] for those packed columns."""
            mk = mkp.tile([NCLS + 1, 512], F16, tag="mk", name="mk")
            nc.vector.tensor_mul(
                mk[:, 0:length],
                q[0 : NCLS + 1, bank_off : bank_off + length],
                cm[:, colbuf_lo : colbuf_lo + length],
            )
            stile = stp.tile([1, 512], F32, tag="stile", name="stile")
            nc.tensor.matmul(
                stile[0:1, 0:length],
                lhsT=ones[:],
                rhs=mk[:, 0:length],
                start=True,
                stop=True,
                skip_group_check=True,
            )
            nc.vector.tensor_copy(
                fs[:, colbuf_lo : colbuf_lo + length],
                q[0:1, bank_off : bank_off + length],
            )
            nc.vector.tensor_copy(
                fs[:, NCOL + colbuf_lo : NCOL + colbuf_lo + length],
                stile[0:1, 0:length],
            )

        # ---- octant phase: full square, column-side only ----
        with tc_ctx.tile_pool(name="qo", bufs=2, space="PSUM") as qop:
            qo = [qop.tile([128, 512], F32, tag="qo", name=f"qo{h}") for h in (0, 1)]
            for r8 in range(8):
                z = zp.tile([128, 1024], F32, tag="z", name="zo")
                for h in (0, 1):
                    nc.tensor.matmul(
                        z[:, 512 * h : 512 * h + 512],
                        lhsT=boot[:, 128 * r8 : 128 * r8 + 128],
                        rhs=boot[:, 512 * h : 512 * h + 512],
                        start=True,
                        stop=True,
                        skip_group_check=True,
                    )
                ez = ezp.tile([128, 1024], F16, tag="ez", name="ezo")
                nc.scalar.activation(
                    ez[:], z[:], mybir.ActivationFunctionType.Exp,
                    scale=1.0 / TEMPERATURE,
                )
                for h in (0, 1):
                    nc.tensor.matmul(
                        qo[h][:, :],
                        lhsT=boot[:, 1024 + 128 * r8 : 1024 + 128 * r8 + 128],
                        rhs=ez[:, 512 * h : 512 * h + 512],
                        start=(r8 == 0),
                        stop=(r8 == 7),
                        skip_group_check=True,
                    )
            extract(qo[0], 0, 1536, 512)
            extract(qo[1], 0, 2048, 512)

        # ---- main phase ----
        with tc_ctx.tile_pool(name="qm", bufs=3, space="PSUM") as qmp:
            qbank = {
                n: qmp.tile([128, 512], F32, tag="qm", name=f"q_{n}")
                for n in ("q64", "q32", "q16")
            }
            for batches_w, exts_w in WINDOWS:
              for batch in batches_w:
                z = zp.tile([128, 1024], F32, tag="z", name="zm")
                spans = []
                off = 0
                for r in batch:
                    lo, hi = _span(r)
                    for a, b in _split512(off, off + (hi - lo)):
                        nc.tensor.matmul(
                            z[:, a:b],
                            lhsT=xm[:, 128 * r : 128 * r + 128],
                            rhs=colsm[:, lo + a - off : lo + b - off],
                            start=True,
                            stop=True,
                            skip_group_check=True,
                        )
                    spans.append((r, lo, hi, off))
                    off += hi - lo
                assert off <= 1024
                ez = ezp.tile([128, 1024], F16, tag="ez", name="ezm")
                nc.scalar.activation(
                    ez[:, 0:off], z[:, 0:off], mybir.ActivationFunctionType.Exp,
                    scale=1.0 / TEMPERATURE,
                )
                for r, lo, hi, off_r in spans:
                    for name in _active_segs(r):
                        slo, ln, bank, boff, _, _ = SEGS[name]
                        nc.tensor.matmul(
                            qbank[bank][:, boff : boff + ln],
                            lhsT=ohm[:, 128 * r : 128 * r + 128],
                            rhs=ez[:, off_r + slo - lo : off_r + slo - lo + ln],
                            start=(r == fl[name][0]),
                            stop=(r == fl[name][1]),
                            skip_group_check=True,
                        )
                # row reductions run in place on the ez tile: the full-sum
                # tensor_scalar multiplies by 1.0 (values preserved), the
                # masked scalar_tensor_tensor destroys its span and so runs
                # last.  No scratch buffers needed.
                if "norow" in _KDBG:
                    continue
                if "nots" not in _KDBG:
                  for r, lo, hi, off_r in spans:
                    ez_r = ez[:, off_r : off_r + (hi - lo)]
                    nc.vector.tensor_scalar(
                        ez_r,
                        ez_r,
                        1.0,
                        None,
                        op0=ALU.mult,
                        op1=ALU.add,
                        accum_out=rw[:, r : r + 1],
                    )
                if "nostt" in _KDBG:
                    continue
                for r, lo, hi, off_r in spans:
                    L = hi - lo
                    ez_r = ez[:, off_r : off_r + L]
                    # masked row sums (same-class); TensorScalarPtr is only
                    # legal on DVE (walrus rejects it on Pool/GpSimd)
                    nc.vector.scalar_tensor_tensor(
                        ez_r,
                        tcb[:, lo:hi],
                        trb[:, r : r + 1],
                        ez_r,
                        op0=ALU.is_equal,
                        op1=ALU.mult,
                        accum_out=rw[:, 56 + r : 57 + r],
                    )
              for n in exts_w:
                s = SEGS[n]
                extract(qbank[s[2]], s[3], s[0], s[1])

        # results out on the SWDGE queue
        nc.gpsimd.dma_start(fs_d[:], fs[:])
        nc.gpsimd.dma_start(rw_d[:], rw[:])

    _strip_self_engine_waits(nc)
    return nc


def _split_drain_waits(nc: bass.Bass, max_waits: int = 1) -> None:
    """walrus codegen caps sync waits per instruction; the kernel-tail drain
    waits on all processors. Split its wait list across a chain of preceding
    drains on the same engine."""
    for bb in nc.main_func.blocks:
        out = []
        for ins in bb.instructions:
            si = ins.sync_info
            waits = list(si.on_wait) if si and si.on_wait else []
            if type(ins).__name__ == "InstDrain" and len(waits) > max_waits:
                chunks = [
                    waits[i : i + max_waits] for i in range(0, len(waits), max_waits)
                ]
                for j, ch in enumerate(chunks[:-1]):
                    out.append(
                        mybir.InstDrain(
                            name=f"{ins.name}-w{j}",
                            ins=[],
                            outs=[],
                            engine=ins.engine,
                            sync_info=mybir.SyncInfo(on_wait=ch, on_update=[]),
                        )
                    )
                ins.sync_info = mybir.SyncInfo(
                    on_wait=chunks[-1], on_update=list(si.on_update or [])
                )
            out.append(ins)
        bb.instructions[:] = out


def _strip_self_engine_waits(nc: bass.Bass) -> None:
    """Drop semaphore waits an engine instruction holds on its *own* engine's
    semaphore when it also waits on another engine (walrus rejects >1 sync
    wait on compute-engine instructions). Engines execute in order, so a
    self-engine wait is always satisfied by program order."""
    prefix = {
        mybir.EngineType.Activation: "Activation_",
        mybir.EngineType.PE: "PE_",
        mybir.EngineType.DVE: "DVE_",
        mybir.EngineType.Pool: "Pool_",
    }
    for bb in nc.main_func.blocks:
        for ins in bb.instructions:
            si = ins.sync_info
            if not si or not si.on_wait or len(si.on_wait) < 2:
                continue
            pref = prefix.get(ins.engine)
            if pref is None:
                continue
            kept = [w for w in si.on_wait if not (w.ant_name or "").startswith(pref)]
            if len(kept) != len(si.on_wait):
                ins.sync_info = mybir.SyncInfo(
                    on_wait=kept, on_update=list(si.on_update)
                )


def _hoist_excess_waits(nc: bass.Bass) -> None:
    """Walrus rejects >1 sync wait on compute-engine instructions.  Split
    the excess onto same-engine drains inserted IMMEDIATELY before the
    owner: nothing executes between drain and owner on that engine, so
    this is exactly equivalent to the original multi-wait (deadlock-free
    by construction, unlike hoisting onto distant carriers, which the
    TileScheduler's reordering can turn into a semaphore cycle)."""
    compute = {
        mybir.EngineType.Activation,
        mybir.EngineType.PE,
        mybir.EngineType.DVE,
        mybir.EngineType.Pool,
    }
    for bb in nc.main_func.blocks:
        out = []
        for ins in bb.instructions:
            si = ins.sync_info
            waits = list(si.on_wait) if si and si.on_wait else []
            if ins.engine in compute and len(waits) > 1:
                for j, w in enumerate(waits[:-1]):
                    out.append(
                        mybir.InstDrain(
                            name=f"{ins.name}-hw{j}",
                            ins=[],
                            outs=[],
                            engine=ins.engine,
                            sync_info=mybir.SyncInfo(on_wait=[w], on_update=[]),
                        )
                    )
                ins.sync_info = mybir.SyncInfo(
                    on_wait=waits[-1:], on_update=list(si.on_update)
                )
            out.append(ins)
        bb.instructions[:] = out


def _get_program(split_waits: bool = True) -> bass.Bass:
    global _PROGRAM, _PROGRAM_SPLIT
    if _PROGRAM is None:
        _PROGRAM = _build_program()
        _PROGRAM_SPLIT = False
    if split_waits and not _PROGRAM_SPLIT:
        _hoist_excess_waits(_PROGRAM)
        _split_drain_waits(_PROGRAM)
        _PROGRAM_SPLIT = True
    return _PROGRAM


def _core_blocks(c):
    """packed column layout of core c as global 128-col block indices."""
    return (
        [8 + c, 16 + 2 * c, 17 + 2 * c]
        + [32 + 4 * c + k for k in range(4)]
        + [24 + c, 40 + c, 48 + 2 * c, 49 + 2 * c, 56 + c]
        + list(range(8 * c, 8 * c + 8))
    )


def _prepare_in_maps(out_1, out_2, target):
    x = np.concatenate(
        [np.asarray(out_1, np.float32), np.asarray(out_2, np.float32)], axis=0
    )
    xt = np.ascontiguousarray(x.astype(np.float16).T)  # [128, 8192]
    t2 = np.concatenate([np.asarray(target), np.asarray(target)]).astype(np.int64)

    oh = np.zeros((TWO_B, 128), np.float16)
    oh[:, 0] = 1.0                      # ones column -> colQ row 0 = colfull
    oh[np.arange(TWO_B), 1 + t2] = 1.0
    # block-major: [128 rows-of-block (partitions), 56*128 (block, class)]
    ohm = np.ascontiguousarray(
        oh[:NSEG].reshape(56, 128, 128).transpose(1, 0, 2).reshape(128, NSEG)
    )
    xm = np.ascontiguousarray(xt[:, :NSEG])
    tr = np.ascontiguousarray(t2[:NSEG].reshape(56, 128).T.astype(np.float32))

    in_maps = []
    for c in range(N_CORES):
        blocks = _core_blocks(c)
        colidx = np.concatenate([np.arange(128 * b, 128 * b + 128) for b in blocks])
        cols = np.ascontiguousarray(xt[:, colidx])  # [128, 2560]
        oho = np.ascontiguousarray(
            oh[1024 * c : 1024 * c + 1024]
            .reshape(8, 128, 128)
            .transpose(1, 0, 2)
            .reshape(128, 1024)
        )
        boot = np.ascontiguousarray(
            np.concatenate([cols[:, 1536:2560], oho], axis=1)
        )
        tcols = t2[colidx]
        cmask = (
            np.arange(NCLS + 1, dtype=np.int64)[:, None] == (1 + tcols)[None, :]
        ).astype(np.float32)
        tcb = np.ascontiguousarray(
            np.broadcast_to(tcols.astype(np.float16)[None, :], (128, NCOL))
        )
        in_maps.append(
            {
                "boot": boot,
                "colsm": np.ascontiguousarray(cols[:, :1536]),
                "xm": xm,
                "cm": cmask,
                "ohm": ohm,
                "tc": tcb,
                "tr": tr,
            }
        )
    return in_maps


def _finish(fs_per_core, rw_per_core) -> np.ndarray:
    full = np.zeros(TWO_B, np.float64)
    s = np.zeros(TWO_B, np.float64)
    for c in range(N_CORES):
        blocks = _core_blocks(c)
        colidx = np.concatenate([np.arange(128 * b, 128 * b + 128) for b in blocks])
        fsc = np.asarray(fs_per_core[c], np.float64).reshape(-1)
        np.add.at(full, colidx, fsc[:NCOL])
        np.add.at(s, colidx, fsc[NCOL:])
        rwc = np.asarray(rw_per_core[c], np.float64)  # [128, 112]
        full[:NSEG] += rwc[:, :56].T.reshape(-1)
        s[:NSEG] += rwc[:, 56:].T.reshape(-1)
    n = TWO_B - 2
    ng = full - s
    o1 = full - (1.0 - TAU_PLUS) * ng
    o2 = full + (n * TAU_PLUS - (1.0 - TAU_PLUS)) * ng
    loss = float(np.mean(np.log(o2) - np.log(o1)))
    return np.array(loss, dtype=np.float32)


def run(out_1, out_2, out_m, target, trace=False):
    """Run on hardware; returns (loss, exec_time_ns or None)."""
    nc = _get_program()
    in_maps = _prepare_in_maps(out_1, out_2, target)
    res = run_bass_kernel_spmd(nc, in_maps, list(range(N_CORES)), trace=trace)
    fs = [res.results[i]["fs"] for i in range(N_CORES)]
    rw = [res.results[i]["rw"] for i in range(N_CORES)]
    return _finish(fs, rw), res.exec_time_ns


def kernel(out_1, out_2, out_m, target):
    loss, _ = run(out_1, out_2, out_m, target, trace=False)
    return loss


# revision 27
# speedup vs baseline: 1.0127x; 1.0127x over previous
"""DebiasedPosLossV2 contrastive loss on 8 Trainium2 NeuronCores.

Math (reference, B=4096, D=128, TEMP=0.5, TAU=0.1):
    out = concat([out_1, out_2])            # [2B, D], rows L2-normalized
    ez  = exp(out @ out.T / TEMP)           # [2B, 2B], symmetric
    full_i = sum_j ez_ij
    S_i    = sum_{j: t_j == t_i} ez_ij      (t = concat([target, target]))
    Ng = full - S;  loss = mean(log(o2) - log(o1))

ez is symmetric, so only the upper block-triangle is computed (plus the
block diagonal), cutting PE matmul and ScalarE exp work roughly in half
versus the full matrix.  For a computed element (i <= j):
  - its contribution to column j (full_j / S_j) comes from a one-hot
    class matmul on the TensorEngine (column side),
  - its contribution to ROW i (= the missing lower part of column i, by
    symmetry) comes from free-dim reductions on the otherwise-idle
    Vector/GpSimd engines: tensor_scalar with accum_out for full,
    scalar_tensor_tensor (mask-and-multiply-and-sum) for S.
The host sums the per-core column-side and row-side partials.

SPMD uniformity: run_bass_kernel_spmd runs ONE program on all 8 cores,
so every core must execute an identical instruction stream.  The upper
triangle (64x64 blocks of 128) is decomposed recursively:
  Q64:  rows  0-31 x col-blocks 32-63  -> each core a 512-col strip
  Q32:  rows  0-15 x blocks 16-31, rows 32-47 x blocks 48-63 -> 256 cols
  Q16:  4 instances of rows (8k..8k+7) x next 8 blocks -> 128 cols each
  octant: blocks 8c..8c+7 square, computed FULLY, column-side only
    (a symmetric square's column sums already count every ordered pair).
Every core gets: a 56-iteration main loop over global row-blocks 0..55
with a band-contiguous packed column buffer, plus an 8-iteration octant
loop.  All geometry is core-independent; only host-packed data differs.
Per-core work: 288 [128,128] blocks (vs 512 for the full matrix).
"""

import os
import sys

if "/opt/trn_rl_repo" not in sys.path:
    sys.path.insert(0, "/opt/trn_rl_repo")

_KDBG = set(os.environ.get("KDBG", "").split(","))

from contextlib import ExitStack

import numpy as np

import concourse.bass as bass
import concourse.mybir as mybir
import concourse.tile as tile
from concourse.bass_utils import run_bass_kernel_spmd

B = 4096
D = 128
TWO_B = 2 * B
TEMPERATURE = 0.5
TAU_PLUS = 0.1
N_CORES = 8
NCLS = 100
NSEG = 56 * 128          # main-loop rows (global rows 0..7167)
NCOL = 2560              # packed columns per core (1536 main + 1024 octant)

F16 = mybir.dt.float16
F32 = mybir.dt.float32

# main-loop segments: name -> (colbuf_lo, length, q_bank, bank_off, r_lo, r_hi)
SEGS = {
    "q16_1": (0, 128, "q16", 0, 0, 8),
    "q32_p": (128, 256, "q32", 0, 0, 16),
    "q64": (384, 512, "q64", 0, 0, 32),
    "q16_2": (896, 128, "q16", 128, 16, 24),
    "q16_3": (1024, 128, "q16", 256, 32, 40),
    "q32_r": (1152, 256, "q32", 256, 32, 48),
    "q16_4": (1408, 128, "q16", 384, 48, 56),
}

# A matmul with start=True resets the whole PSUM bank, so segments sharing
# a bank must be strictly sequential: windows of rows, each followed by the
# extract of any segment that just finished.  Within a window, rows are
# batched into <=1024-column exp groups.
WINDOWS = [
    ([[r] for r in range(0, 8)], ["q16_1"]),            # spans 896
    ([[r] for r in range(8, 16)], ["q32_p"]),           # spans 768
    ([[r] for r in range(16, 24)], ["q16_2"]),          # spans 640
    ([[24, 25], [26, 27], [28, 29], [30, 31]], ["q64"]),     # 512+512
    ([[32, 33], [34, 35], [36, 37], [38, 39]], ["q16_3"]),   # 384+384
    ([[40, 41, 42, 43], [44, 45, 46, 47]], ["q32_r"]),       # 256 x4
    ([list(range(48, 56))], ["q16_4"]),                      # 128 x8
]
BATCHES = [b for w, _ in WINDOWS for b in w]

_PROGRAM = None
_PROGRAM_SPLIT = False


def _active_segs(r):
    return [k for k, s in SEGS.items() if s[4] <= r < s[5]]


def _span(r):
    segs = [SEGS[k] for k in _active_segs(r)]
    lo = min(s[0] for s in segs)
    hi = max(s[0] + s[1] for s in segs)
    assert hi - lo == sum(s[1] for s in segs), f"span not contiguous at r={r}"
    return lo, hi


def _seg_first_last():
    """first/last processed row per segment, in BATCHES order."""
    order = [r for b in BATCHES for r in b]
    fl = {}
    for name, s in SEGS.items():
        rs = [r for r in order if s[4] <= r < s[5]]
        fl[name] = (rs[0], rs[-1])
    return fl


def _split512(a, b):
    """split [a,b) at 512 boundaries (PSUM bank bounds)."""
    out = []
    while a < b:
        nxt = min(b, (a // 512 + 1) * 512)
        out.append((a, nxt))
        a = nxt
    return out


def _build_program() -> bass.Bass:
    nc = bass.Bass()

    # boot: everything the octant phase needs, one DMA: [oct cols | oct oh]
    boot_d = nc.declare_dram_parameter("boot", [128, 2048], F16, isOutput=False)
    colsm_d = nc.declare_dram_parameter("colsm", [128, 1536], F16, isOutput=False)
    xm_d = nc.declare_dram_parameter("xm", [128, NSEG], F16, isOutput=False)
    cm_d = nc.declare_dram_parameter("cm", [NCLS + 1, NCOL], F32, isOutput=False)
    ohm_d = nc.declare_dram_parameter("ohm", [128, NSEG], F16, isOutput=False)
    tc_d = nc.declare_dram_parameter("tc", [128, NCOL], F16, isOutput=False)
    tr_d = nc.declare_dram_parameter("tr", [128, 56], F32, isOutput=False)
    fs_d = nc.declare_dram_parameter("fs", [1, 2 * NCOL], F32, isOutput=True)
    rw_d = nc.declare_dram_parameter("rw", [128, 112], F32, isOutput=True)

    fl = _seg_first_last()
    ALU = mybir.AluOpType

    with ExitStack() as ctx:
        tc_ctx = ctx.enter_context(tile.TileContext(nc))
        const = ctx.enter_context(tc_ctx.tile_pool(name="const", bufs=1))
        # ez tiles are NEVER reused (fits SBUF): exp carries a single PE
        # wait instead of cross-engine pool-rotation waits
        ezp = ctx.enter_context(tc_ctx.tile_pool(name="ez", bufs=44))
        zp = ctx.enter_context(tc_ctx.tile_pool(name="z", bufs=2, space="PSUM"))
        stp = ctx.enter_context(tc_ctx.tile_pool(name="st", bufs=1, space="PSUM"))

        # input DMAs ordered by first use; compute-path tensors share the
        # sync queue (one semaphore), DVE-consumed tensors go on the vector
        # queue so their waits land on DVE touchers
        boot = const.tile([128, 2048], F16, tag="boot")
        nc.sync.dma_start(boot[:], boot_d[:])
        colsm = const.tile([128, 1536], F16, tag="colsm")
        nc.sync.dma_start(colsm[:], colsm_d[:])
        xm = const.tile([128, NSEG], F16, tag="xm")
        nc.sync.dma_start(xm[:], xm_d[:])
        ohm = const.tile([128, NSEG], F16, tag="ohm")
        nc.sync.dma_start(ohm[:], ohm_d[:])
        cm = const.tile([NCLS + 1, NCOL], F32, tag="cm")
        nc.scalar.dma_start(cm[:], cm_d[:])
        tcb = const.tile([128, NCOL], F16, tag="tcb")
        nc.scalar.dma_start(tcb[:], tc_d[:])
        trb = const.tile([128, 56], F32, tag="trb")
        nc.scalar.dma_start(trb[:], tr_d[:])

        ones = const.tile([NCLS + 1, 1], F16, tag="ones")
        nc.gpsimd.memset(ones[:], 1.0)
        fs = const.tile([1, 2 * NCOL], F32, tag="fs")
        rw = const.tile([128, 112], F32, tag="rw")
        if _KDBG & {"norow", "nots", "nostt"}:
            nc.vector.memset(rw[:], 0.0)
        mkp = ctx.enter_context(tc_ctx.tile_pool(name="mkp", bufs=2))
        tch = const.tile([1, 8], F32, tag="tch")         # toucher scratch

        # warm the exp activation table while boot DMA streams
        nc.scalar.activation(
            tch[0:1, 4:5], ones[0:1, 0:1], mybir.ActivationFunctionType.Exp
        )
        # touchers: absorb DMA waits on cheap ops so the real consumers
        # carry a single sync wait each (walrus limit)
        nc.vector.tensor_copy(tch[0:1, 0:1], cm[0:1, 0:1])
        nc.vector.tensor_copy(tch[0:1, 1:2], tcb[0:1, 0:1])
        nc.vector.tensor_copy(tch[0:1, 2:3], trb[0:1, 0:1])

        def extract(q, bank_off, colbuf_lo, length):
            """colQ bank slice -> fs[full | S] for those packed columns."""
            mk = mkp.tile([NCLS + 1, 512], F16, tag="mk", name="mk")
            nc.vector.tensor_mul(
                mk[:, 0:length],
                q[0 : NCLS + 1, bank_off : bank_off + length],
                cm[:, colbuf_lo : colbuf_lo + length],
            )
            stile = stp.tile([1, 512], F32, tag="stile", name="stile")
            nc.tensor.matmul(
                stile[0:1, 0:length],
                lhsT=ones[:],
                rhs=mk[:, 0:length],
                start=True,
                stop=True,
                skip_group_check=True,
            )
            nc.vector.tensor_copy(
                fs[:, colbuf_lo : colbuf_lo + length],
                q[0:1, bank_off : bank_off + length],
            )
            nc.vector.tensor_copy(
                fs[:, NCOL + colbuf_lo : NCOL + colbuf_lo + length],
                stile[0:1, 0:length],
            )

        # ---- octant phase: full square, column-side only ----
        with tc_ctx.tile_pool(name="qo", bufs=2, space="PSUM") as qop:
            qo = [qop.tile([128, 512], F32, tag="qo", name=f"qo{h}") for h in (0, 1)]
            for r8 in range(8):
                z = zp.tile([128, 1024], F32, tag="z", name="zo")
                for h in (0, 1):
                    nc.tensor.matmul(
                        z[:, 512 * h : 512 * h + 512],
                        lhsT=boot[:, 128 * r8 : 128 * r8 + 128],
                        rhs=boot[:, 512 * h : 512 * h + 512],
                        start=True,
                        stop=True,
                        skip_group_check=True,
                    )
                ez = ezp.tile([128, 1024], F16, tag="ez", name="ezo")
                nc.scalar.activation(
                    ez[:], z[:], mybir.ActivationFunctionType.Exp,
                    scale=1.0 / TEMPERATURE,
                )
                for h in (0, 1):
                    nc.tensor.matmul(
                        qo[h][:, :],
                        lhsT=boot[:, 1024 + 128 * r8 : 1024 + 128 * r8 + 128],
                        rhs=ez[:, 512 * h : 512 * h + 512],
                        start=(r8 == 0),
                        stop=(r8 == 7),
                        skip_group_check=True,
                    )
            extract(qo[0], 0, 1536, 512)
            extract(qo[1], 0, 2048, 512)

        # ---- main phase ----
        with tc_ctx.tile_pool(name="qm", bufs=3, space="PSUM") as qmp:
            qbank = {
                n: qmp.tile([128, 512], F32, tag="qm", name=f"q_{n}")
                for n in ("q64", "q32", "q16")
            }
            for batches_w, exts_w in WINDOWS:
              for batch in batches_w:
                z = zp.tile([128, 1024], F32, tag="z", name="zm")
                spans = []
                off = 0
                for r in batch:
                    lo, hi = _span(r)
                    for a, b in _split512(off, off + (hi - lo)):
                        nc.tensor.matmul(
                            z[:, a:b],
                            lhsT=xm[:, 128 * r : 128 * r + 128],
                            rhs=colsm[:, lo + a - off : lo + b - off],
                            start=True,
                            stop=True,
                            skip_group_check=True,
                        )
                    spans.append((r, lo, hi, off))
                    off += hi - lo
                assert off <= 1024
                ez = ezp.tile([128, 1024], F16, tag="ez", name="ezm")
                # single-span batches (wide spans, W1-W3): the exp's own
                # accumulator yields the full row sums for free
                acc = (
                    {"accum_out": rw[:, spans[0][0] : spans[0][0] + 1]}
                    if len(spans) == 1 and "norow" not in _KDBG
                    else {}
                )
                nc.scalar.activation(
                    ez[:, 0:off], z[:, 0:off], mybir.ActivationFunctionType.Exp,
                    scale=1.0 / TEMPERATURE, **acc,
                )
                for r, lo, hi, off_r in spans:
                    for name in _active_segs(r):
                        slo, ln, bank, boff, _, _ = SEGS[name]
                        nc.tensor.matmul(
                            qbank[bank][:, boff : boff + ln],
                            lhsT=ohm[:, 128 * r : 128 * r + 128],
                            rhs=ez[:, off_r + slo - lo : off_r + slo - lo + ln],
                            start=(r == fl[name][0]),
                            stop=(r == fl[name][1]),
                            skip_group_check=True,
                        )
                if "norow" in _KDBG:
                    continue
                # full row sums for multi-span batches: tensor_reduce
                # (single output, no elementwise write)
                if "nots" not in _KDBG and len(spans) > 1:
                    for r, lo, hi, off_r in spans:
                        nc.vector.tensor_reduce(
                            rw[:, r : r + 1],
                            ez[:, off_r : off_r + (hi - lo)],
                            axis=mybir.AxisListType.X,
                            op=ALU.add,
                        )
                if "nostt" in _KDBG:
                    continue
                # masked row sums (same-class), in place on ez (its last
                # reader); TensorScalarPtr is DVE-only (walrus rejects Pool)
                for r, lo, hi, off_r in spans:
                    L = hi - lo
                    ez_r = ez[:, off_r : off_r + L]
                    nc.vector.scalar_tensor_tensor(
                        ez_r,
                        tcb[:, lo:hi],
                        trb[:, r : r + 1],
                        ez_r,
                        op0=ALU.is_equal,
                        op1=ALU.mult,
                        accum_out=rw[:, 56 + r : 57 + r],
                    )
              for n in exts_w:
                s = SEGS[n]
                extract(qbank[s[2]], s[3], s[0], s[1])

        # results out on the SWDGE queue
        nc.gpsimd.dma_start(fs_d[:], fs[:])
        nc.gpsimd.dma_start(rw_d[:], rw[:])

    _strip_self_engine_waits(nc)
    return nc


def _split_drain_waits(nc: bass.Bass, max_waits: int = 1) -> None:
    """walrus codegen caps sync waits per instruction; the kernel-tail drain
    waits on all processors. Split its wait list across a chain of preceding
    drains on the same engine."""
    for bb in nc.main_func.blocks:
        out = []
        for ins in bb.instructions:
            si = ins.sync_info
            waits = list(si.on_wait) if si and si.on_wait else []
            if type(ins).__name__ == "InstDrain" and len(waits) > max_waits:
                chunks = [
                    waits[i : i + max_waits] for i in range(0, len(waits), max_waits)
                ]
                for j, ch in enumerate(chunks[:-1]):
                    out.append(
                        mybir.InstDrain(
                            name=f"{ins.name}-w{j}",
                            ins=[],
                            outs=[],
                            engine=ins.engine,
                            sync_info=mybir.SyncInfo(on_wait=ch, on_update=[]),
                        )
                    )
                ins.sync_info = mybir.SyncInfo(
                    on_wait=chunks[-1], on_update=list(si.on_update or [])
                )
            out.append(ins)
        bb.instructions[:] = out


def _strip_self_engine_waits(nc: bass.Bass) -> None:
    """Drop semaphore waits an engine instruction holds on its *own* engine's
    semaphore when it also waits on another engine (walrus rejects >1 sync
    wait on compute-engine instructions). Engines execute in order, so a
    self-engine wait is always satisfied by program order."""
    prefix = {
        mybir.EngineType.Activation: "Activation_",
        mybir.EngineType.PE: "PE_",
        mybir.EngineType.DVE: "DVE_",
        mybir.EngineType.Pool: "Pool_",
    }
    for bb in nc.main_func.blocks:
        for ins in bb.instructions:
            si = ins.sync_info
            if not si or not si.on_wait or len(si.on_wait) < 2:
                continue
            pref = prefix.get(ins.engine)
            if pref is None:
                continue
            kept = [w for w in si.on_wait if not (w.ant_name or "").startswith(pref)]
            if len(kept) != len(si.on_wait):
                ins.sync_info = mybir.SyncInfo(
                    on_wait=kept, on_update=list(si.on_update)
                )


def _hoist_excess_waits(nc: bass.Bass) -> None:
    """Walrus rejects >1 sync wait on compute-engine instructions.  Split
    the excess onto same-engine drains inserted IMMEDIATELY before the
    owner: nothing executes between drain and owner on that engine, so
    this is exactly equivalent to the original multi-wait (deadlock-free
    by construction, unlike hoisting onto distant carriers, which the
    TileScheduler's reordering can turn into a semaphore cycle)."""
    compute = {
        mybir.EngineType.Activation,
        mybir.EngineType.PE,
        mybir.EngineType.DVE,
        mybir.EngineType.Pool,
    }
    for bb in nc.main_func.blocks:
        out = []
        for ins in bb.instructions:
            si = ins.sync_info
            waits = list(si.on_wait) if si and si.on_wait else []
            if ins.engine in compute and len(waits) > 1:
                for j, w in enumerate(waits[:-1]):
                    out.append(
                        mybir.InstDrain(
                            name=f"{ins.name}-hw{j}",
                            ins=[],
                            outs=[],
                            engine=ins.engine,
                            sync_info=mybir.SyncInfo(on_wait=[w], on_update=[]),
                        )
                    )
                ins.sync_info = mybir.SyncInfo(
                    on_wait=waits[-1:], on_update=list(si.on_update)
                )
            out.append(ins)
        bb.instructions[:] = out


def _ap_key(ap):
    return (str(getattr(ap, "memref", "")), getattr(ap, "offset", None),
            str(getattr(ap, "ap", "")), str(getattr(ap, "dtype", "")))


def _dedup_ldweights(nc: bass.Bass) -> None:
    """The PE array keeps its stationary weights until the next LDWEIGHTS,
    so an InstLdweights identical to the previous one (in the FINAL,
    post-schedule order) is redundant.  Replace it with a cheap PE drain
    carrying the same sync_info (semaphore counts must not change)."""
    for bb in nc.main_func.blocks:
        out = []
        last = None
        for ins in bb.instructions:
            if ins.engine == mybir.EngineType.PE:
                t = type(ins).__name__
                if t == "InstLdweights":
                    key = _ap_key(ins.ins[0]) if ins.ins else None
                    if key is not None and key == last:
                        out.append(
                            mybir.InstDrain(
                                name=f"{ins.name}-dup",
                                ins=[],
                                outs=[],
                                engine=mybir.EngineType.PE,
                                sync_info=ins.sync_info,
                            )
                        )
                        continue
                    last = key
                elif t == "InstMatmult":
                    if getattr(ins, "is_transpose", False):
                        last = None
                elif t not in ("InstEventSemaphore", "InstDrain", "InstNop"):
                    last = None
            out.append(ins)
        bb.instructions[:] = out


def _get_program(split_waits: bool = True) -> bass.Bass:
    global _PROGRAM, _PROGRAM_SPLIT
    if _PROGRAM is None:
        _PROGRAM = _build_program()
        _PROGRAM_SPLIT = False
    if split_waits and not _PROGRAM_SPLIT:
        _dedup_ldweights(_PROGRAM)
        _hoist_excess_waits(_PROGRAM)
        _split_drain_waits(_PROGRAM)
        _PROGRAM_SPLIT = True
    return _PROGRAM


def _core_blocks(c):
    """packed column layout of core c as global 128-col block indices."""
    return (
        [8 + c, 16 + 2 * c, 17 + 2 * c]
        + [32 + 4 * c + k for k in range(4)]
        + [24 + c, 40 + c, 48 + 2 * c, 49 + 2 * c, 56 + c]
        + list(range(8 * c, 8 * c + 8))
    )


def _prepare_in_maps(out_1, out_2, target):
    x = np.concatenate(
        [np.asarray(out_1, np.float32), np.asarray(out_2, np.float32)], axis=0
    )
    xt = np.ascontiguousarray(x.astype(np.float16).T)  # [128, 8192]
    t2 = np.concatenate([np.asarray(target), np.asarray(target)]).astype(np.int64)

    oh = np.zeros((TWO_B, 128), np.float16)
    oh[:, 0] = 1.0                      # ones column -> colQ row 0 = colfull
    oh[np.arange(TWO_B), 1 + t2] = 1.0
    # block-major: [128 rows-of-block (partitions), 56*128 (block, class)]
    ohm = np.ascontiguousarray(
        oh[:NSEG].reshape(56, 128, 128).transpose(1, 0, 2).reshape(128, NSEG)
    )
    xm = np.ascontiguousarray(xt[:, :NSEG])
    tr = np.ascontiguousarray(t2[:NSEG].reshape(56, 128).T.astype(np.float32))

    in_maps = []
    for c in range(N_CORES):
        blocks = _core_blocks(c)
        colidx = np.concatenate([np.arange(128 * b, 128 * b + 128) for b in blocks])
        cols = np.ascontiguousarray(xt[:, colidx])  # [128, 2560]
        oho = np.ascontiguousarray(
            oh[1024 * c : 1024 * c + 1024]
            .reshape(8, 128, 128)
            .transpose(1, 0, 2)
            .reshape(128, 1024)
        )
        boot = np.ascontiguousarray(
            np.concatenate([cols[:, 1536:2560], oho], axis=1)
        )
        tcols = t2[colidx]
        cmask = (
            np.arange(NCLS + 1, dtype=np.int64)[:, None] == (1 + tcols)[None, :]
        ).astype(np.float32)
        tcb = np.ascontiguousarray(
            np.broadcast_to(tcols.astype(np.float16)[None, :], (128, NCOL))
        )
        in_maps.append(
            {
                "boot": boot,
                "colsm": np.ascontiguousarray(cols[:, :1536]),
                "xm": xm,
                "cm": cmask,
                "ohm": ohm,
                "tc": tcb,
                "tr": tr,
            }
        )
    return in_maps


def _finish(fs_per_core, rw_per_core) -> np.ndarray:
    full = np.zeros(TWO_B, np.float64)
    s = np.zeros(TWO_B, np.float64)
    for c in range(N_CORES):
        blocks = _core_blocks(c)
        colidx = np.concatenate([np.arange(128 * b, 128 * b + 128) for b in blocks])
        fsc = np.asarray(fs_per_core[c], np.float64).reshape(-1)
        np.add.at(full, colidx, fsc[:NCOL])
        np.add.at(s, colidx, fsc[NCOL:])
        rwc = np.asarray(rw_per_core[c], np.float64)  # [128, 112]
        full[:NSEG] += rwc[:, :56].T.reshape(-1)
        s[:NSEG] += rwc[:, 56:].T.reshape(-1)
    n = TWO_B - 2
    ng = full - s
    o1 = full - (1.0 - TAU_PLUS) * ng
    o2 = full + (n * TAU_PLUS - (1.0 - TAU_PLUS)) * ng
    loss = float(np.mean(np.log(o2) - np.log(o1)))
    return np.array(loss, dtype=np.float32)


def run(out_1, out_2, out_m, target, trace=False):
    """Run on hardware; returns (loss, exec_time_ns or None)."""
    nc = _get_program()
    in_maps = _prepare_in_maps(out_1, out_2, target)
    res = run_bass_kernel_spmd(nc, in_maps, list(range(N_CORES)), trace=trace)
    fs = [res.results[i]["fs"] for i in range(N_CORES)]
    rw = [res.results[i]["rw"] for i in range(N_CORES)]
    return _finish(fs, rw), res.exec_time_ns


def kernel(out_1, out_2, out_m, target):
    loss, _ = run(out_1, out_2, out_m, target, trace=False)
    return loss


# revision 28
# speedup vs baseline: 1.1884x; 1.1735x over previous
"""DebiasedPosLossV2 contrastive loss on 8 Trainium2 NeuronCores.

Math (reference, B=4096, D=128, TEMP=0.5, TAU=0.1):
    out = concat([out_1, out_2])            # [2B, D], rows L2-normalized
    ez  = exp(out @ out.T / TEMP)           # [2B, 2B], symmetric
    full_i = sum_j ez_ij
    S_i    = sum_{j: t_j == t_i} ez_ij      (t = concat([target, target]))
    Ng = full - S;  loss = mean(log(o2) - log(o1))

ez is symmetric, so only the upper block-triangle is computed (plus the
block diagonal), cutting PE matmul and ScalarE exp work roughly in half
versus the full matrix.  For a computed element (i <= j):
  - its contribution to column j (full_j / S_j) comes from a one-hot
    class matmul on the TensorEngine (column side),
  - its contribution to ROW i (= the missing lower part of column i, by
    symmetry) comes from free-dim reductions on the otherwise-idle
    Vector/GpSimd engines: tensor_scalar with accum_out for full,
    scalar_tensor_tensor (mask-and-multiply-and-sum) for S.
The host sums the per-core column-side and row-side partials.

SPMD uniformity: run_bass_kernel_spmd runs ONE program on all 8 cores,
so every core must execute an identical instruction stream.  The upper
triangle (64x64 blocks of 128) is decomposed recursively:
  Q64:  rows  0-31 x col-blocks 32-63  -> each core a 512-col strip
  Q32:  rows  0-15 x blocks 16-31, rows 32-47 x blocks 48-63 -> 256 cols
  Q16:  4 instances of rows (8k..8k+7) x next 8 blocks -> 128 cols each
  octant: blocks 8c..8c+7 square, computed FULLY, column-side only
    (a symmetric square's column sums already count every ordered pair).
Every core gets: a 56-iteration main loop over global row-blocks 0..55
with a band-contiguous packed column buffer, plus an 8-iteration octant
loop.  All geometry is core-independent; only host-packed data differs.
Per-core work: 288 [128,128] blocks (vs 512 for the full matrix).
"""

import os
import sys

if "/opt/trn_rl_repo" not in sys.path:
    sys.path.insert(0, "/opt/trn_rl_repo")

_KDBG = set(os.environ.get("KDBG", "").split(","))

from contextlib import ExitStack

import numpy as np

import concourse.bass as bass
import concourse.mybir as mybir
import concourse.tile as tile
from concourse.bass_utils import run_bass_kernel_spmd

B = 4096
D = 128
TWO_B = 2 * B
TEMPERATURE = 0.5
TAU_PLUS = 0.1
N_CORES = 8
NCLS = 100
NSEG = 56 * 128          # main-loop rows (global rows 0..7167)
NCOL = 2560              # packed columns per core (1536 main + 1024 octant)

F16 = mybir.dt.float16
F32 = mybir.dt.float32

# main-loop segments: name -> (colbuf_lo, length, q_bank, bank_off, r_lo, r_hi)
SEGS = {
    "q16_1": (0, 128, "q16", 0, 0, 8),
    "q32_p": (128, 256, "q32", 0, 0, 16),
    "q64": (384, 512, "q64", 0, 0, 32),
    "q16_2": (896, 128, "q16", 128, 16, 24),
    "q16_3": (1024, 128, "q16", 256, 32, 40),
    "q32_r": (1152, 256, "q32", 256, 32, 48),
    "q16_4": (1408, 128, "q16", 384, 48, 56),
}

# A matmul with start=True resets the whole PSUM bank, so segments sharing
# a bank must be strictly sequential: windows of rows, each followed by the
# extract of any segment that just finished.  Within a window, rows are
# batched into <=1024-column exp groups.
WINDOWS = [
    ([[r] for r in range(0, 8)], ["q16_1"]),            # spans 896
    ([[r] for r in range(8, 16)], ["q32_p"]),           # spans 768
    ([[r] for r in range(16, 24)], ["q16_2"]),          # spans 640
    ([[24, 25], [26, 27], [28, 29], [30, 31]], ["q64"]),     # 512+512
    ([[32, 33], [34, 35], [36, 37], [38, 39]], ["q16_3"]),   # 384+384
    ([[40, 41, 42, 43], [44, 45, 46, 47]], ["q32_r"]),       # 256 x4
    ([list(range(48, 56))], ["q16_4"]),                      # 128 x8
]
BATCHES = [b for w, _ in WINDOWS for b in w]

_PROGRAM = None
_PROGRAM_SPLIT = False


def _active_segs(r):
    return [k for k, s in SEGS.items() if s[4] <= r < s[5]]


def _span(r):
    segs = [SEGS[k] for k in _active_segs(r)]
    lo = min(s[0] for s in segs)
    hi = max(s[0] + s[1] for s in segs)
    assert hi - lo == sum(s[1] for s in segs), f"span not contiguous at r={r}"
    return lo, hi


def _seg_first_last():
    """first/last processed row per segment, in BATCHES order."""
    order = [r for b in BATCHES for r in b]
    fl = {}
    for name, s in SEGS.items():
        rs = [r for r in order if s[4] <= r < s[5]]
        fl[name] = (rs[0], rs[-1])
    return fl


def _split512(a, b):
    """split [a,b) at 512 boundaries (PSUM bank bounds)."""
    out = []
    while a < b:
        nxt = min(b, (a // 512 + 1) * 512)
        out.append((a, nxt))
        a = nxt
    return out


def _build_program() -> bass.Bass:
    nc = bass.Bass()

    # boot: everything the octant phase needs, one DMA: [oct cols | oct oh]
    boot_d = nc.declare_dram_parameter("boot", [128, 2048], F16, isOutput=False)
    colsm_d = nc.declare_dram_parameter("colsm", [128, 1536], F16, isOutput=False)
    xm_d = nc.declare_dram_parameter("xm", [128, NSEG], F16, isOutput=False)
    cm_d = nc.declare_dram_parameter("cm", [NCLS + 1, NCOL], F32, isOutput=False)
    ohm_d = nc.declare_dram_parameter("ohm", [128, NSEG], F16, isOutput=False)
    tc_d = nc.declare_dram_parameter("tc", [128, NCOL], F16, isOutput=False)
    tr_d = nc.declare_dram_parameter("tr", [128, 56], F32, isOutput=False)
    fs_d = nc.declare_dram_parameter("fs", [1, 2 * NCOL], F32, isOutput=True)
    rw_d = nc.declare_dram_parameter("rw", [128, 112], F32, isOutput=True)

    fl = _seg_first_last()
    ALU = mybir.AluOpType

    with ExitStack() as ctx:
        tc_ctx = ctx.enter_context(tile.TileContext(nc))
        const = ctx.enter_context(tc_ctx.tile_pool(name="const", bufs=1))
        # ez tiles are NEVER reused (fits SBUF): exp carries a single PE
        # wait instead of cross-engine pool-rotation waits
        ezp = ctx.enter_context(tc_ctx.tile_pool(name="ez", bufs=44))
        zp = ctx.enter_context(tc_ctx.tile_pool(name="z", bufs=2, space="PSUM"))
        stp = ctx.enter_context(tc_ctx.tile_pool(name="st", bufs=1, space="PSUM"))

        # input DMAs ordered by first use; compute-path tensors share the
        # sync queue (one semaphore), DVE-consumed tensors go on the vector
        # queue so their waits land on DVE touchers
        boot = const.tile([128, 2048], F16, tag="boot")
        nc.sync.dma_start(boot[:], boot_d[:])
        colsm = const.tile([128, 1536], F16, tag="colsm")
        nc.sync.dma_start(colsm[:], colsm_d[:])
        xm = const.tile([128, NSEG], F16, tag="xm")
        nc.sync.dma_start(xm[:], xm_d[:])
        ohm = const.tile([128, NSEG], F16, tag="ohm")
        nc.gpsimd.dma_start(ohm[:], ohm_d[:])
        cm = const.tile([NCLS + 1, NCOL], F32, tag="cm")
        nc.gpsimd.dma_start(cm[:], cm_d[:])
        tcb = const.tile([128, NCOL], F16, tag="tcb")
        nc.gpsimd.dma_start(tcb[:], tc_d[:])
        trb = const.tile([128, 56], F32, tag="trb")
        nc.gpsimd.dma_start(trb[:], tr_d[:])

        ones = const.tile([NCLS + 1, 1], F16, tag="ones")
        nc.gpsimd.memset(ones[:], 1.0)
        fs = const.tile([1, 2 * NCOL], F32, tag="fs")
        rw = const.tile([128, 112], F32, tag="rw")
        if _KDBG & {"norow", "nots", "nostt"}:
            nc.vector.memset(rw[:], 0.0)
        mkp = ctx.enter_context(tc_ctx.tile_pool(name="mkp", bufs=2))
        tch = const.tile([1, 8], F32, tag="tch")         # toucher scratch

        # warm the exp activation table while boot DMA streams
        nc.scalar.activation(
            tch[0:1, 4:5], ones[0:1, 0:1], mybir.ActivationFunctionType.Exp
        )
        # touchers: absorb DMA waits on cheap ops so the real consumers
        # carry a single sync wait each (walrus limit)
        nc.vector.tensor_copy(tch[0:1, 0:1], cm[0:1, 0:1])
        nc.vector.tensor_copy(tch[0:1, 1:2], tcb[0:1, 0:1])
        nc.vector.tensor_copy(tch[0:1, 2:3], trb[0:1, 0:1])

        def extract(q, bank_off, colbuf_lo, length):
            """colQ bank slice -> fs[full | S] for those packed columns."""
            mk = mkp.tile([NCLS + 1, 512], F16, tag="mk", name="mk")
            nc.vector.tensor_mul(
                mk[:, 0:length],
                q[0 : NCLS + 1, bank_off : bank_off + length],
                cm[:, colbuf_lo : colbuf_lo + length],
            )
            stile = stp.tile([1, 512], F32, tag="stile", name="stile")
            nc.tensor.matmul(
                stile[0:1, 0:length],
                lhsT=ones[:],
                rhs=mk[:, 0:length],
                start=True,
                stop=True,
                skip_group_check=True,
            )
            nc.vector.tensor_copy(
                fs[:, colbuf_lo : colbuf_lo + length],
                q[0:1, bank_off : bank_off + length],
            )
            nc.vector.tensor_copy(
                fs[:, NCOL + colbuf_lo : NCOL + colbuf_lo + length],
                stile[0:1, 0:length],
            )

        # ---- octant phase: full square, column-side only ----
        with tc_ctx.tile_pool(name="qo", bufs=2, space="PSUM") as qop:
            qo = [qop.tile([128, 512], F32, tag="qo", name=f"qo{h}") for h in (0, 1)]
            for r8 in range(8):
                z = zp.tile([128, 1024], F32, tag="z", name="zo")
                for h in (0, 1):
                    nc.tensor.matmul(
                        z[:, 512 * h : 512 * h + 512],
                        lhsT=boot[:, 128 * r8 : 128 * r8 + 128],
                        rhs=boot[:, 512 * h : 512 * h + 512],
                        start=True,
                        stop=True,
                        skip_group_check=True,
                    )
                ez = ezp.tile([128, 1024], F16, tag="ez", name="ezo")
                nc.scalar.activation(
                    ez[:], z[:], mybir.ActivationFunctionType.Exp,
                    scale=1.0 / TEMPERATURE,
                )
                for h in (0, 1):
                    nc.tensor.matmul(
                        qo[h][:, :],
                        lhsT=boot[:, 1024 + 128 * r8 : 1024 + 128 * r8 + 128],
                        rhs=ez[:, 512 * h : 512 * h + 512],
                        start=(r8 == 0),
                        stop=(r8 == 7),
                        skip_group_check=True,
                    )
            extract(qo[0], 0, 1536, 512)
            extract(qo[1], 0, 2048, 512)

        # ---- main phase ----
        with tc_ctx.tile_pool(name="qm", bufs=3, space="PSUM") as qmp:
            qbank = {
                n: qmp.tile([128, 512], F32, tag="qm", name=f"q_{n}")
                for n in ("q64", "q32", "q16")
            }
            for batches_w, exts_w in WINDOWS:
              for batch in batches_w:
                z = zp.tile([128, 1024], F32, tag="z", name="zm")
                spans = []
                off = 0
                for r in batch:
                    lo, hi = _span(r)
                    for a, b in _split512(off, off + (hi - lo)):
                        nc.tensor.matmul(
                            z[:, a:b],
                            lhsT=xm[:, 128 * r : 128 * r + 128],
                            rhs=colsm[:, lo + a - off : lo + b - off],
                            start=True,
                            stop=True,
                            skip_group_check=True,
                        )
                    spans.append((r, lo, hi, off))
                    off += hi - lo
                assert off <= 1024
                ez = ezp.tile([128, 1024], F16, tag="ez", name="ezm")
                # single-span batches (wide spans, W1-W3): the exp's own
                # accumulator yields the full row sums for free
                acc = (
                    {"accum_out": rw[:, spans[0][0] : spans[0][0] + 1]}
                    if len(spans) == 1 and "norow" not in _KDBG
                    else {}
                )
                nc.scalar.activation(
                    ez[:, 0:off], z[:, 0:off], mybir.ActivationFunctionType.Exp,
                    scale=1.0 / TEMPERATURE, **acc,
                )
                for r, lo, hi, off_r in spans:
                    for name in _active_segs(r):
                        slo, ln, bank, boff, _, _ = SEGS[name]
                        nc.tensor.matmul(
                            qbank[bank][:, boff : boff + ln],
                            lhsT=ohm[:, 128 * r : 128 * r + 128],
                            rhs=ez[:, off_r + slo - lo : off_r + slo - lo + ln],
                            start=(r == fl[name][0]),
                            stop=(r == fl[name][1]),
                            skip_group_check=True,
                        )
                if "norow" in _KDBG:
                    continue
                # full row sums for multi-span batches: tensor_reduce
                # (single output, no elementwise write)
                if "nots" not in _KDBG and len(spans) > 1:
                    for r, lo, hi, off_r in spans:
                        nc.vector.tensor_reduce(
                            rw[:, r : r + 1],
                            ez[:, off_r : off_r + (hi - lo)],
                            axis=mybir.AxisListType.X,
                            op=ALU.add,
                        )
                if "nostt" in _KDBG:
                    continue
                # masked row sums (same-class), in place on ez (its last
                # reader); TensorScalarPtr is DVE-only (walrus rejects Pool)
                for r, lo, hi, off_r in spans:
                    L = hi - lo
                    ez_r = ez[:, off_r : off_r + L]
                    nc.vector.scalar_tensor_tensor(
                        ez_r,
                        tcb[:, lo:hi],
                        trb[:, r : r + 1],
                        ez_r,
                        op0=ALU.is_equal,
                        op1=ALU.mult,
                        accum_out=rw[:, 56 + r : 57 + r],
                    )
              for n in exts_w:
                s = SEGS[n]
                extract(qbank[s[2]], s[3], s[0], s[1])

        # results out on the SWDGE queue
        nc.gpsimd.dma_start(fs_d[:], fs[:])
        nc.gpsimd.dma_start(rw_d[:], rw[:])

    _strip_self_engine_waits(nc)
    return nc


def _split_drain_waits(nc: bass.Bass, max_waits: int = 1) -> None:
    """walrus codegen caps sync waits per instruction; the kernel-tail drain
    waits on all processors. Split its wait list across a chain of preceding
    drains on the same engine."""
    for bb in nc.main_func.blocks:
        out = []
        for ins in bb.instructions:
            si = ins.sync_info
            waits = list(si.on_wait) if si and si.on_wait else []
            if type(ins).__name__ == "InstDrain" and len(waits) > max_waits:
                chunks = [
                    waits[i : i + max_waits] for i in range(0, len(waits), max_waits)
                ]
                for j, ch in enumerate(chunks[:-1]):
                    out.append(
                        mybir.InstDrain(
                            name=f"{ins.name}-w{j}",
                            ins=[],
                            outs=[],
                            engine=ins.engine,
                            sync_info=mybir.SyncInfo(on_wait=ch, on_update=[]),
                        )
                    )
                ins.sync_info = mybir.SyncInfo(
                    on_wait=chunks[-1], on_update=list(si.on_update or [])
                )
            out.append(ins)
        bb.instructions[:] = out


def _strip_self_engine_waits(nc: bass.Bass) -> None:
    """Drop semaphore waits an engine instruction holds on its *own* engine's
    semaphore when it also waits on another engine (walrus rejects >1 sync
    wait on compute-engine instructions). Engines execute in order, so a
    self-engine wait is always satisfied by program order."""
    prefix = {
        mybir.EngineType.Activation: "Activation_",
        mybir.EngineType.PE: "PE_",
        mybir.EngineType.DVE: "DVE_",
        mybir.EngineType.Pool: "Pool_",
    }
    for bb in nc.main_func.blocks:
        for ins in bb.instructions:
            si = ins.sync_info
            if not si or not si.on_wait or len(si.on_wait) < 2:
                continue
            pref = prefix.get(ins.engine)
            if pref is None:
                continue
            kept = [w for w in si.on_wait if not (w.ant_name or "").startswith(pref)]
            if len(kept) != len(si.on_wait):
                ins.sync_info = mybir.SyncInfo(
                    on_wait=kept, on_update=list(si.on_update)
                )


def _hoist_excess_waits(nc: bass.Bass) -> None:
    """Walrus rejects >1 sync wait on compute-engine instructions.  Split
    the excess onto same-engine drains inserted IMMEDIATELY before the
    owner: nothing executes between drain and owner on that engine, so
    this is exactly equivalent to the original multi-wait (deadlock-free
    by construction, unlike hoisting onto distant carriers, which the
    TileScheduler's reordering can turn into a semaphore cycle)."""
    compute = {
        mybir.EngineType.Activation,
        mybir.EngineType.PE,
        mybir.EngineType.DVE,
        mybir.EngineType.Pool,
    }
    for bb in nc.main_func.blocks:
        out = []
        for ins in bb.instructions:
            si = ins.sync_info
            waits = list(si.on_wait) if si and si.on_wait else []
            if ins.engine in compute and len(waits) > 1:
                for j, w in enumerate(waits[:-1]):
                    out.append(
                        mybir.InstNoOp(
                            name=f"{ins.name}-hw{j}",
                            ins=[],
                            outs=[],
                            engine=ins.engine,
                            sync_info=mybir.SyncInfo(on_wait=[w], on_update=[]),
                        )
                    )
                ins.sync_info = mybir.SyncInfo(
                    on_wait=waits[-1:], on_update=list(si.on_update)
                )
            out.append(ins)
        bb.instructions[:] = out


def _ap_key(ap):
    return (str(getattr(ap, "memref", "")), getattr(ap, "offset", None),
            str(getattr(ap, "ap", "")), str(getattr(ap, "dtype", "")))


def _dedup_ldweights(nc: bass.Bass) -> None:
    """The PE array keeps its stationary weights until the next LDWEIGHTS,
    so an InstLdweights identical to the previous one (in the FINAL,
    post-schedule order) is redundant.  Replace it with a cheap PE drain
    carrying the same sync_info (semaphore counts must not change)."""
    for bb in nc.main_func.blocks:
        out = []
        last = None
        for ins in bb.instructions:
            if ins.engine == mybir.EngineType.PE:
                t = type(ins).__name__
                if t == "InstLdweights":
                    key = _ap_key(ins.ins[0]) if ins.ins else None
                    if key is not None and key == last:
                        out.append(
                            mybir.InstNoOp(
                                name=f"{ins.name}-dup",
                                ins=[],
                                outs=[],
                                engine=mybir.EngineType.PE,
                                sync_info=ins.sync_info,
                            )
                        )
                        continue
                    last = key
                elif t == "InstMatmult":
                    if getattr(ins, "is_transpose", False):
                        last = None
                elif t not in ("InstEventSemaphore", "InstDrain", "InstNop"):
                    last = None
            out.append(ins)
        bb.instructions[:] = out


def _get_program(split_waits: bool = True) -> bass.Bass:
    global _PROGRAM, _PROGRAM_SPLIT
    if _PROGRAM is None:
        _PROGRAM = _build_program()
        _PROGRAM_SPLIT = False
    if split_waits and not _PROGRAM_SPLIT:
        _dedup_ldweights(_PROGRAM)
        _hoist_excess_waits(_PROGRAM)
        _split_drain_waits(_PROGRAM)
        _PROGRAM_SPLIT = True
    return _PROGRAM


def _core_blocks(c):
    """packed column layout of core c as global 128-col block indices."""
    return (
        [8 + c, 16 + 2 * c, 17 + 2 * c]
        + [32 + 4 * c + k for k in range(4)]
        + [24 + c, 40 + c, 48 + 2 * c, 49 + 2 * c, 56 + c]
        + list(range(8 * c, 8 * c + 8))
    )


def _prepare_in_maps(out_1, out_2, target):
    x = np.concatenate(
        [np.asarray(out_1, np.float32), np.asarray(out_2, np.float32)], axis=0
    )
    xt = np.ascontiguousarray(x.astype(np.float16).T)  # [128, 8192]
    t2 = np.concatenate([np.asarray(target), np.asarray(target)]).astype(np.int64)

    oh = np.zeros((TWO_B, 128), np.float16)
    oh[:, 0] = 1.0                      # ones column -> colQ row 0 = colfull
    oh[np.arange(TWO_B), 1 + t2] = 1.0
    # block-major: [128 rows-of-block (partitions), 56*128 (block, class)]
    ohm = np.ascontiguousarray(
        oh[:NSEG].reshape(56, 128, 128).transpose(1, 0, 2).reshape(128, NSEG)
    )
    xm = np.ascontiguousarray(xt[:, :NSEG])
    tr = np.ascontiguousarray(t2[:NSEG].reshape(56, 128).T.astype(np.float32))

    in_maps = []
    for c in range(N_CORES):
        blocks = _core_blocks(c)
        colidx = np.concatenate([np.arange(128 * b, 128 * b + 128) for b in blocks])
        cols = np.ascontiguousarray(xt[:, colidx])  # [128, 2560]
        oho = np.ascontiguousarray(
            oh[1024 * c : 1024 * c + 1024]
            .reshape(8, 128, 128)
            .transpose(1, 0, 2)
            .reshape(128, 1024)
        )
        boot = np.ascontiguousarray(
            np.concatenate([cols[:, 1536:2560], oho], axis=1)
        )
        tcols = t2[colidx]
        cmask = (
            np.arange(NCLS + 1, dtype=np.int64)[:, None] == (1 + tcols)[None, :]
        ).astype(np.float32)
        tcb = np.ascontiguousarray(
            np.broadcast_to(tcols.astype(np.float16)[None, :], (128, NCOL))
        )
        in_maps.append(
            {
                "boot": boot,
                "colsm": np.ascontiguousarray(cols[:, :1536]),
                "xm": xm,
                "cm": cmask,
                "ohm": ohm,
                "tc": tcb,
                "tr": tr,
            }
        )
    return in_maps


def _finish(fs_per_core, rw_per_core) -> np.ndarray:
    full = np.zeros(TWO_B, np.float64)
    s = np.zeros(TWO_B, np.float64)
    for c in range(N_CORES):
        blocks = _core_blocks(c)
        colidx = np.concatenate([np.arange(128 * b, 128 * b + 128) for b in blocks])
        fsc = np.asarray(fs_per_core[c], np.float64).reshape(-1)
        np.add.at(full, colidx, fsc[:NCOL])
        np.add.at(s, colidx, fsc[NCOL:])
        rwc = np.asarray(rw_per_core[c], np.float64)  # [128, 112]
        full[:NSEG] += rwc[:, :56].T.reshape(-1)
        s[:NSEG] += rwc[:, 56:].T.reshape(-1)
    n = TWO_B - 2
    ng = full - s
    o1 = full - (1.0 - TAU_PLUS) * ng
    o2 = full + (n * TAU_PLUS - (1.0 - TAU_PLUS)) * ng
    loss = float(np.mean(np.log(o2) - np.log(o1)))
    return np.array(loss, dtype=np.float32)


def run(out_1, out_2, out_m, target, trace=False):
    """Run on hardware; returns (loss, exec_time_ns or None)."""
    nc = _get_program()
    in_maps = _prepare_in_maps(out_1, out_2, target)
    res = run_bass_kernel_spmd(nc, in_maps, list(range(N_CORES)), trace=trace)
    fs = [res.results[i]["fs"] for i in range(N_CORES)]
    rw = [res.results[i]["rw"] for i in range(N_CORES)]
    return _finish(fs, rw), res.exec_time_ns


def kernel(out_1, out_2, out_m, target):
    loss, _ = run(out_1, out_2, out_m, target, trace=False)
    return loss


# revision 29
# speedup vs baseline: 1.6233x; 1.3659x over previous
"""DebiasedPosLossV2 contrastive loss on 8 Trainium2 NeuronCores.

Math (reference, B=4096, D=128, TEMP=0.5, TAU=0.1):
    out = concat([out_1, out_2])            # [2B, D], rows L2-normalized
    ez  = exp(out @ out.T / TEMP)           # [2B, 2B], symmetric
    full_i = sum_j ez_ij
    S_i    = sum_{j: t_j == t_i} ez_ij      (t = concat([target, target]))
    Ng = full - S;  loss = mean(log(o2) - log(o1))

ez is symmetric, so only the upper block-triangle is computed (plus the
block diagonal), cutting PE matmul and ScalarE exp work roughly in half
versus the full matrix.  For a computed element (i <= j):
  - its contribution to column j (full_j / S_j) comes from a one-hot
    class matmul on the TensorEngine (column side),
  - its contribution to ROW i (= the missing lower part of column i, by
    symmetry) comes from free-dim reductions on the otherwise-idle
    Vector/GpSimd engines: tensor_scalar with accum_out for full,
    scalar_tensor_tensor (mask-and-multiply-and-sum) for S.
The host sums the per-core column-side and row-side partials.

SPMD uniformity: run_bass_kernel_spmd runs ONE program on all 8 cores,
so every core must execute an identical instruction stream.  The upper
triangle (64x64 blocks of 128) is decomposed recursively:
  Q64:  rows  0-31 x col-blocks 32-63  -> each core a 512-col strip
  Q32:  rows  0-15 x blocks 16-31, rows 32-47 x blocks 48-63 -> 256 cols
  Q16:  4 instances of rows (8k..8k+7) x next 8 blocks -> 128 cols each
  octant: blocks 8c..8c+7 square, computed FULLY, column-side only
    (a symmetric square's column sums already count every ordered pair).
Every core gets: a 56-iteration main loop over global row-blocks 0..55
with a band-contiguous packed column buffer, plus an 8-iteration octant
loop.  All geometry is core-independent; only host-packed data differs.
Per-core work: 288 [128,128] blocks (vs 512 for the full matrix).
"""

import os
import sys

if "/opt/trn_rl_repo" not in sys.path:
    sys.path.insert(0, "/opt/trn_rl_repo")

_KDBG = set(os.environ.get("KDBG", "").split(","))

from contextlib import ExitStack

import numpy as np

import concourse.bass as bass
import concourse.mybir as mybir
import concourse.tile as tile
from concourse.bass_utils import run_bass_kernel_spmd

B = 4096
D = 128
TWO_B = 2 * B
TEMPERATURE = 0.5
TAU_PLUS = 0.1
N_CORES = 8
NCLS = 100
NSEG = 56 * 128          # main-loop rows (global rows 0..7167)
NCOL = 2560              # packed columns per core (1536 main + 1024 octant)

F16 = mybir.dt.float16
F32 = mybir.dt.float32

# main-loop segments: name -> (colbuf_lo, length, q_bank, bank_off, r_lo, r_hi)
SEGS = {
    "q16_1": (0, 128, "q16", 0, 0, 8),
    "q32_p": (128, 256, "q32", 0, 0, 16),
    "q64": (384, 512, "q64", 0, 0, 32),
    "q16_2": (896, 128, "q16", 128, 16, 24),
    "q16_3": (1024, 128, "q16", 256, 32, 40),
    "q32_r": (1152, 256, "q32", 256, 32, 48),
    "q16_4": (1408, 128, "q16", 384, 48, 56),
}

# A matmul with start=True resets the whole PSUM bank, so segments sharing
# a bank must be strictly sequential: windows of rows, each followed by the
# extract of any segment that just finished.  Within a window, rows are
# batched into <=1024-column exp groups.
WINDOWS = [
    ([[r] for r in range(0, 8)], ["q16_1"]),            # spans 896
    ([[r] for r in range(8, 16)], ["q32_p"]),           # spans 768
    ([[r] for r in range(16, 24)], ["q16_2"]),          # spans 640
    ([[24, 25], [26, 27], [28, 29], [30, 31]], ["q64"]),     # 512+512
    ([[32, 33], [34, 35], [36, 37], [38, 39]], ["q16_3"]),   # 384+384
    ([[40, 41, 42, 43], [44, 45, 46, 47]], ["q32_r"]),       # 256 x4
    ([list(range(48, 56))], ["q16_4"]),                      # 128 x8
]
BATCHES = [b for w, _ in WINDOWS for b in w]

_PROGRAM = None
_PROGRAM_SPLIT = False


def _active_segs(r):
    return [k for k, s in SEGS.items() if s[4] <= r < s[5]]


def _span(r):
    segs = [SEGS[k] for k in _active_segs(r)]
    lo = min(s[0] for s in segs)
    hi = max(s[0] + s[1] for s in segs)
    assert hi - lo == sum(s[1] for s in segs), f"span not contiguous at r={r}"
    return lo, hi


def _seg_first_last():
    """first/last processed row per segment, in BATCHES order."""
    order = [r for b in BATCHES for r in b]
    fl = {}
    for name, s in SEGS.items():
        rs = [r for r in order if s[4] <= r < s[5]]
        fl[name] = (rs[0], rs[-1])
    return fl


def _split512(a, b):
    """split [a,b) at 512 boundaries (PSUM bank bounds)."""
    out = []
    while a < b:
        nxt = min(b, (a // 512 + 1) * 512)
        out.append((a, nxt))
        a = nxt
    return out


def _build_program() -> bass.Bass:
    nc = bass.Bass()

    # boot: everything the octant phase needs, one DMA: [oct cols | oct oh]
    boot_d = nc.declare_dram_parameter("boot", [128, 2048], F16, isOutput=False)
    colsm_d = nc.declare_dram_parameter("colsm", [128, 1536], F16, isOutput=False)
    xm_d = nc.declare_dram_parameter("xm", [128, NSEG], F16, isOutput=False)
    cm_d = nc.declare_dram_parameter("cm", [128, NCOL], F16, isOutput=False)
    ohm_d = nc.declare_dram_parameter("ohm", [128, NSEG], F16, isOutput=False)
    tc_d = nc.declare_dram_parameter("tc", [128, NCOL], F16, isOutput=False)
    tr_d = nc.declare_dram_parameter("tr", [128, 56], F32, isOutput=False)
    fs_d = nc.declare_dram_parameter("fs", [1, 2 * NCOL], F32, isOutput=True)
    rw_d = nc.declare_dram_parameter("rw", [128, 112], F32, isOutput=True)

    fl = _seg_first_last()
    ALU = mybir.AluOpType

    with ExitStack() as ctx:
        tc_ctx = ctx.enter_context(tile.TileContext(nc))
        const = ctx.enter_context(tc_ctx.tile_pool(name="const", bufs=1))
        # ez tiles are NEVER reused (fits SBUF): exp carries a single PE
        # wait instead of cross-engine pool-rotation waits
        ezp = ctx.enter_context(tc_ctx.tile_pool(name="ez", bufs=44))
        zp = ctx.enter_context(tc_ctx.tile_pool(name="z", bufs=2, space="PSUM"))
        stp = ctx.enter_context(tc_ctx.tile_pool(name="st", bufs=1, space="PSUM"))

        # input DMAs ordered by first use; compute-path tensors share the
        # sync queue (one semaphore), DVE-consumed tensors go on the vector
        # queue so their waits land on DVE touchers
        boot = const.tile([128, 2048], F16, tag="boot")
        nc.sync.dma_start(boot[:], boot_d[:])
        colsm = const.tile([128, 1536], F16, tag="colsm")
        nc.sync.dma_start(colsm[:], colsm_d[:])
        xm = const.tile([128, NSEG], F16, tag="xm")
        nc.sync.dma_start(xm[:], xm_d[:])
        ohm = const.tile([128, NSEG], F16, tag="ohm")
        nc.sync.dma_start(ohm[:], ohm_d[:])
        cm = const.tile([128, NCOL], F16, tag="cm")
        nc.sync.dma_start(cm[:], cm_d[:])
        tcb = const.tile([128, NCOL], F16, tag="tcb")
        nc.sync.dma_start(tcb[:], tc_d[:])
        trb = const.tile([128, 56], F32, tag="trb")
        nc.sync.dma_start(trb[:], tr_d[:])

        ones = const.tile([NCLS + 1, 1], F16, tag="ones")
        nc.gpsimd.memset(ones[:], 1.0)
        fs = const.tile([1, 2 * NCOL], F32, tag="fs")
        rw = const.tile([128, 112], F32, tag="rw")
        if _KDBG & {"norow", "nots", "nostt"}:
            nc.vector.memset(rw[:], 0.0)
        mkp = ctx.enter_context(tc_ctx.tile_pool(name="mkp", bufs=2))
        tch = const.tile([1, 8], F32, tag="tch")         # toucher scratch

        # warm the exp activation table while boot DMA streams
        nc.scalar.activation(
            tch[0:1, 4:5], ones[0:1, 0:1], mybir.ActivationFunctionType.Exp
        )
        # touchers: absorb DMA waits on cheap ops so the real consumers
        # carry a single sync wait each (walrus limit)
        nc.vector.tensor_copy(tch[0:1, 0:1], cm[0:1, 0:1])
        nc.vector.tensor_copy(tch[0:1, 1:2], tcb[0:1, 0:1])
        nc.vector.tensor_copy(tch[0:1, 2:3], trb[0:1, 0:1])

        def extract(q, bank_off, colbuf_lo, length):
            """colQ bank slice -> fs[full | S] for those packed columns."""
            mk = mkp.tile([NCLS + 1, 512], F16, tag="mk", name="mk")
            nc.vector.tensor_mul(
                mk[:, 0:length],
                q[0 : NCLS + 1, bank_off : bank_off + length],
                cm[0 : NCLS + 1, colbuf_lo : colbuf_lo + length],
            )
            stile = stp.tile([1, 512], F32, tag="stile", name="stile")
            nc.tensor.matmul(
                stile[0:1, 0:length],
                lhsT=ones[:],
                rhs=mk[:, 0:length],
                start=True,
                stop=True,
                skip_group_check=True,
            )
            nc.vector.tensor_copy(
                fs[:, colbuf_lo : colbuf_lo + length],
                q[0:1, bank_off : bank_off + length],
            )
            nc.vector.tensor_copy(
                fs[:, NCOL + colbuf_lo : NCOL + colbuf_lo + length],
                stile[0:1, 0:length],
            )

        # ---- octant phase: full square, column-side only ----
        with tc_ctx.tile_pool(name="qo", bufs=2, space="PSUM") as qop:
            qo = [qop.tile([128, 512], F32, tag="qo", name=f"qo{h}") for h in (0, 1)]
            for r8 in range(8):
                z = zp.tile([128, 1024], F32, tag="z", name="zo")
                for h in (0, 1):
                    nc.tensor.matmul(
                        z[:, 512 * h : 512 * h + 512],
                        lhsT=boot[:, 128 * r8 : 128 * r8 + 128],
                        rhs=boot[:, 512 * h : 512 * h + 512],
                        start=True,
                        stop=True,
                        skip_group_check=True,
                    )
                ez = ezp.tile([128, 1024], F16, tag="ez", name="ezo")
                nc.scalar.activation(
                    ez[:], z[:], mybir.ActivationFunctionType.Exp,
                    scale=1.0 / TEMPERATURE,
                )
                for h in (0, 1):
                    nc.tensor.matmul(
                        qo[h][:, :],
                        lhsT=boot[:, 1024 + 128 * r8 : 1024 + 128 * r8 + 128],
                        rhs=ez[:, 512 * h : 512 * h + 512],
                        start=(r8 == 0),
                        stop=(r8 == 7),
                        skip_group_check=True,
                    )
            extract(qo[0], 0, 1536, 512)
            extract(qo[1], 0, 2048, 512)

        # ---- main phase ----
        with tc_ctx.tile_pool(name="qm", bufs=3, space="PSUM") as qmp:
            qbank = {
                n: qmp.tile([128, 512], F32, tag="qm", name=f"q_{n}")
                for n in ("q64", "q32", "q16")
            }
            for batches_w, exts_w in WINDOWS:
              for batch in batches_w:
                z = zp.tile([128, 1024], F32, tag="z", name="zm")
                spans = []
                off = 0
                for r in batch:
                    lo, hi = _span(r)
                    for a, b in _split512(off, off + (hi - lo)):
                        nc.tensor.matmul(
                            z[:, a:b],
                            lhsT=xm[:, 128 * r : 128 * r + 128],
                            rhs=colsm[:, lo + a - off : lo + b - off],
                            start=True,
                            stop=True,
                            skip_group_check=True,
                        )
                    spans.append((r, lo, hi, off))
                    off += hi - lo
                assert off <= 1024
                ez = ezp.tile([128, 1024], F16, tag="ez", name="ezm")
                # single-span batches (wide spans, W1-W3): the exp's own
                # accumulator yields the full row sums for free
                acc = (
                    {"accum_out": rw[:, spans[0][0] : spans[0][0] + 1]}
                    if len(spans) == 1 and "norow" not in _KDBG
                    else {}
                )
                nc.scalar.activation(
                    ez[:, 0:off], z[:, 0:off], mybir.ActivationFunctionType.Exp,
                    scale=1.0 / TEMPERATURE, **acc,
                )
                for r, lo, hi, off_r in spans:
                    for name in _active_segs(r):
                        slo, ln, bank, boff, _, _ = SEGS[name]
                        nc.tensor.matmul(
                            qbank[bank][:, boff : boff + ln],
                            lhsT=ohm[:, 128 * r : 128 * r + 128],
                            rhs=ez[:, off_r + slo - lo : off_r + slo - lo + ln],
                            start=(r == fl[name][0]),
                            stop=(r == fl[name][1]),
                            skip_group_check=True,
                        )
                if "norow" in _KDBG:
                    continue
                # full row sums for multi-span batches: tensor_reduce
                # (single output, no elementwise write)
                if "nots" not in _KDBG and len(spans) > 1:
                    for r, lo, hi, off_r in spans:
                        nc.vector.tensor_reduce(
                            rw[:, r : r + 1],
                            ez[:, off_r : off_r + (hi - lo)],
                            axis=mybir.AxisListType.X,
                            op=ALU.add,
                        )
                if "nostt" in _KDBG:
                    continue
                # masked row sums (same-class), in place on ez (its last
                # reader); TensorScalarPtr is DVE-only (walrus rejects Pool)
                for r, lo, hi, off_r in spans:
                    L = hi - lo
                    ez_r = ez[:, off_r : off_r + L]
                    nc.vector.scalar_tensor_tensor(
                        ez_r,
                        tcb[:, lo:hi],
                        trb[:, r : r + 1],
                        ez_r,
                        op0=ALU.is_equal,
                        op1=ALU.mult,
                        accum_out=rw[:, 56 + r : 57 + r],
                    )
              for n in exts_w:
                s = SEGS[n]
                extract(qbank[s[2]], s[3], s[0], s[1])

        # results out on the SWDGE queue
        nc.gpsimd.dma_start(fs_d[:], fs[:])
        nc.gpsimd.dma_start(rw_d[:], rw[:])

    _strip_self_engine_waits(nc)
    return nc


def _split_drain_waits(nc: bass.Bass, max_waits: int = 1) -> None:
    """walrus codegen caps sync waits per instruction; the kernel-tail drain
    waits on all processors. Split its wait list across a chain of preceding
    drains on the same engine."""
    for bb in nc.main_func.blocks:
        out = []
        for ins in bb.instructions:
            si = ins.sync_info
            waits = list(si.on_wait) if si and si.on_wait else []
            if type(ins).__name__ == "InstDrain" and len(waits) > max_waits:
                chunks = [
                    waits[i : i + max_waits] for i in range(0, len(waits), max_waits)
                ]
                for j, ch in enumerate(chunks[:-1]):
                    out.append(
                        mybir.InstDrain(
                            name=f"{ins.name}-w{j}",
                            ins=[],
                            outs=[],
                            engine=ins.engine,
                            sync_info=mybir.SyncInfo(on_wait=ch, on_update=[]),
                        )
                    )
                ins.sync_info = mybir.SyncInfo(
                    on_wait=chunks[-1], on_update=list(si.on_update or [])
                )
            out.append(ins)
        bb.instructions[:] = out


def _strip_self_engine_waits(nc: bass.Bass) -> None:
    """Drop semaphore waits an engine instruction holds on its *own* engine's
    semaphore when it also waits on another engine (walrus rejects >1 sync
    wait on compute-engine instructions). Engines execute in order, so a
    self-engine wait is always satisfied by program order."""
    prefix = {
        mybir.EngineType.Activation: "Activation_",
        mybir.EngineType.PE: "PE_",
        mybir.EngineType.DVE: "DVE_",
        mybir.EngineType.Pool: "Pool_",
    }
    for bb in nc.main_func.blocks:
        for ins in bb.instructions:
            si = ins.sync_info
            if not si or not si.on_wait or len(si.on_wait) < 2:
                continue
            pref = prefix.get(ins.engine)
            if pref is None:
                continue
            kept = [w for w in si.on_wait if not (w.ant_name or "").startswith(pref)]
            if len(kept) != len(si.on_wait):
                ins.sync_info = mybir.SyncInfo(
                    on_wait=kept, on_update=list(si.on_update)
                )


def _hoist_excess_waits(nc: bass.Bass) -> None:
    """Walrus rejects >1 sync wait on compute-engine instructions.  Split
    the excess onto same-engine drains inserted IMMEDIATELY before the
    owner: nothing executes between drain and owner on that engine, so
    this is exactly equivalent to the original multi-wait (deadlock-free
    by construction, unlike hoisting onto distant carriers, which the
    TileScheduler's reordering can turn into a semaphore cycle)."""
    compute = {
        mybir.EngineType.Activation,
        mybir.EngineType.PE,
        mybir.EngineType.DVE,
        mybir.EngineType.Pool,
    }
    for bb in nc.main_func.blocks:
        out = []
        for ins in bb.instructions:
            si = ins.sync_info
            waits = list(si.on_wait) if si and si.on_wait else []
            if ins.engine in compute and len(waits) > 1:
                for j, w in enumerate(waits[:-1]):
                    out.append(
                        mybir.InstNoOp(
                            name=f"{ins.name}-hw{j}",
                            ins=[],
                            outs=[],
                            engine=ins.engine,
                            sync_info=mybir.SyncInfo(on_wait=[w], on_update=[]),
                        )
                    )
                ins.sync_info = mybir.SyncInfo(
                    on_wait=waits[-1:], on_update=list(si.on_update)
                )
            out.append(ins)
        bb.instructions[:] = out


def _ap_key(ap):
    return (str(getattr(ap, "memref", "")), getattr(ap, "offset", None),
            str(getattr(ap, "ap", "")), str(getattr(ap, "dtype", "")))


def _dedup_ldweights(nc: bass.Bass) -> None:
    """The PE array keeps its stationary weights until the next LDWEIGHTS,
    so an InstLdweights identical to the previous one (in the FINAL,
    post-schedule order) is redundant.  Replace it with a cheap PE drain
    carrying the same sync_info (semaphore counts must not change)."""
    for bb in nc.main_func.blocks:
        out = []
        last = None
        for ins in bb.instructions:
            if ins.engine == mybir.EngineType.PE:
                t = type(ins).__name__
                if t == "InstLdweights":
                    key = _ap_key(ins.ins[0]) if ins.ins else None
                    if key is not None and key == last:
                        out.append(
                            mybir.InstNoOp(
                                name=f"{ins.name}-dup",
                                ins=[],
                                outs=[],
                                engine=mybir.EngineType.PE,
                                sync_info=ins.sync_info,
                            )
                        )
                        continue
                    last = key
                elif t == "InstMatmult":
                    if getattr(ins, "is_transpose", False):
                        last = None
                elif t not in ("InstEventSemaphore", "InstDrain", "InstNop"):
                    last = None
            out.append(ins)
        bb.instructions[:] = out


def _get_program(split_waits: bool = True) -> bass.Bass:
    global _PROGRAM, _PROGRAM_SPLIT
    if _PROGRAM is None:
        _PROGRAM = _build_program()
        _PROGRAM_SPLIT = False
    if split_waits and not _PROGRAM_SPLIT:
        _dedup_ldweights(_PROGRAM)
        _hoist_excess_waits(_PROGRAM)
        _split_drain_waits(_PROGRAM)
        _PROGRAM_SPLIT = True
    return _PROGRAM


def _core_blocks(c):
    """packed column layout of core c as global 128-col block indices."""
    return (
        [8 + c, 16 + 2 * c, 17 + 2 * c]
        + [32 + 4 * c + k for k in range(4)]
        + [24 + c, 40 + c, 48 + 2 * c, 49 + 2 * c, 56 + c]
        + list(range(8 * c, 8 * c + 8))
    )


def _prepare_in_maps(out_1, out_2, target):
    x = np.concatenate(
        [np.asarray(out_1, np.float32), np.asarray(out_2, np.float32)], axis=0
    )
    xt = np.ascontiguousarray(x.astype(np.float16).T)  # [128, 8192]
    t2 = np.concatenate([np.asarray(target), np.asarray(target)]).astype(np.int64)

    oh = np.zeros((TWO_B, 128), np.float16)
    oh[:, 0] = 1.0                      # ones column -> colQ row 0 = colfull
    oh[np.arange(TWO_B), 1 + t2] = 1.0
    # block-major: [128 rows-of-block (partitions), 56*128 (block, class)]
    ohm = np.ascontiguousarray(
        oh[:NSEG].reshape(56, 128, 128).transpose(1, 0, 2).reshape(128, NSEG)
    )
    xm = np.ascontiguousarray(xt[:, :NSEG])
    tr = np.ascontiguousarray(t2[:NSEG].reshape(56, 128).T.astype(np.float32))

    in_maps = []
    for c in range(N_CORES):
        blocks = _core_blocks(c)
        colidx = np.concatenate([np.arange(128 * b, 128 * b + 128) for b in blocks])
        cols = np.ascontiguousarray(xt[:, colidx])  # [128, 2560]
        oho = np.ascontiguousarray(
            oh[1024 * c : 1024 * c + 1024]
            .reshape(8, 128, 128)
            .transpose(1, 0, 2)
            .reshape(128, 1024)
        )
        boot = np.ascontiguousarray(
            np.concatenate([cols[:, 1536:2560], oho], axis=1)
        )
        tcols = t2[colidx]
        cmask = (
            np.arange(128, dtype=np.int64)[:, None] == (1 + tcols)[None, :]
        ).astype(np.float16)
        tcb = np.ascontiguousarray(
            np.broadcast_to(tcols.astype(np.float16)[None, :], (128, NCOL))
        )
        in_maps.append(
            {
                "boot": boot,
                "colsm": np.ascontiguousarray(cols[:, :1536]),
                "xm": xm,
                "cm": cmask,
                "ohm": ohm,
                "tc": tcb,
                "tr": tr,
            }
        )
    return in_maps


def _finish(fs_per_core, rw_per_core) -> np.ndarray:
    full = np.zeros(TWO_B, np.float64)
    s = np.zeros(TWO_B, np.float64)
    for c in range(N_CORES):
        blocks = _core_blocks(c)
        colidx = np.concatenate([np.arange(128 * b, 128 * b + 128) for b in blocks])
        fsc = np.asarray(fs_per_core[c], np.float64).reshape(-1)
        np.add.at(full, colidx, fsc[:NCOL])
        np.add.at(s, colidx, fsc[NCOL:])
        rwc = np.asarray(rw_per_core[c], np.float64)  # [128, 112]
        full[:NSEG] += rwc[:, :56].T.reshape(-1)
        s[:NSEG] += rwc[:, 56:].T.reshape(-1)
    n = TWO_B - 2
    ng = full - s
    o1 = full - (1.0 - TAU_PLUS) * ng
    o2 = full + (n * TAU_PLUS - (1.0 - TAU_PLUS)) * ng
    loss = float(np.mean(np.log(o2) - np.log(o1)))
    return np.array(loss, dtype=np.float32)


def run(out_1, out_2, out_m, target, trace=False):
    """Run on hardware; returns (loss, exec_time_ns or None)."""
    nc = _get_program()
    in_maps = _prepare_in_maps(out_1, out_2, target)
    res = run_bass_kernel_spmd(nc, in_maps, list(range(N_CORES)), trace=trace)
    fs = [res.results[i]["fs"] for i in range(N_CORES)]
    rw = [res.results[i]["rw"] for i in range(N_CORES)]
    return _finish(fs, rw), res.exec_time_ns


def kernel(out_1, out_2, out_m, target):
    loss, _ = run(out_1, out_2, out_m, target, trace=False)
    return loss
